# revision 31
# baseline (speedup 1.0000x reference)
"""Trainium2 Bass kernel for nn_AddBeta (VAE encoder + parallel-GRU decode + AddBeta).

Sharding: pure data parallel over batch. Each of the 8 cores gets a 64-batch
shard; the tiny encoder MLP (whose BatchNorm uses full-batch statistics) is
computed replicated on every core with the batch ROTATED per core so that each
core's own 64 batches sit in columns 0:64 of the feature-major activations
(BN stats and the latent loss are batch-permutation invariant). No collectives.

Encoder (fp32): x^T chunks stream through 3 matmuls; BN stats come free via
activation accum_out; mu/std/logvar chunks feed z = mu + eps*std (feature-
major), the latent loss (ones-matmul partition reduce), and gh = z @ w_hh.T +
biases (batch-major via PE, then 12 PE transposes give per-batch gh columns).

Decode (fp16, batch-quads of 4): xs is broadcast across partitions once per
quad by a PE outer product (ones x xs -> PSUM -> fp16 SBUF tile). Gate
pre-activations then need NO matmuls: r/z come from ScalarE activations with
per-feature weight as `scale` and per-batch gh column as `bias`; the n-gate
input is a GpSimd tensor_scalar. The GRU combine is fused into per-batch
scalar_tensor_tensor ops on VectorE, tanh/sigmoid on ScalarE (both LUT funcs
share one act table - no table thrash). The output head accumulates 4 feature
chunks in PSUM on top of the AddBeta weights preloaded by an I2 x wpb fp16
matmul; results DMA out as fp16 and are upcast on host.

Engine balance per quad-chunk: ScalarE 8 sigmoid + 1 tanh, VectorE 8 fused
stt + q + relu + evacs, GpSimd gin, PE 2 bcast + 8+2 head matmuls.
"""

import numpy as np

# ---- problem constants (hardcoded; kernel.py must be self-contained) ----
BS = 512          # total batch
NCORES = 8
BSH = BS // NCORES  # 64 batches per core
T = 256           # output length
H = 512           # latent
HC = H // 128     # 4 feature chunks
PAIRS = BSH // 2  # 32
BN_EPS = 1e-5
T_CROSS = 20

# prow packed-row offsets (free dim of a [2, NROW] f32 tensor)
WPB_OFF = 0                 # [2,1024]: AddBeta weights + fc_b, tiled x4
BSUM_OFF = WPB_OFF + 4 * T  # [1,1536]: b_ih+b_hh (r,z) / b_hh (n) in row0
ONES_OFF = BSUM_OFF + 3 * H  # [1,64]: ones in row0
NROW = ONES_OFF + BSH

# pcol packed-column indices ([128, NCOL] f32)
C_B1, C_G1, C_BB1, C_B2, C_G2, C_BB2 = 0, 1, 2, 3, 4, 5
C_B3MU = 6    # 6..9
C_HB3LV = 10  # 10..13
C_B3LV = 14   # 14..17
C_ONES = 18
C_FCW = 19    # 19+2c, 20+2c for chunk c
C_EPS = 27
C_N256 = 28
C_WR = 29     # 29..32: w_ih r-gate chunk cols
C_WZ = 33     # 33..36: z-gate
C_WN = 37     # 37..40: n-gate
C_BIHN = 41   # 41..44: b_ih n-gate
NCOL = 45


def _build_program():
    import concourse.bass as bass
    import concourse.bacc as bacc
    import concourse.tile as tile
    from concourse import mybir
    from contextlib import ExitStack

    f32 = mybir.dt.float32
    AF = mybir.ActivationFunctionType
    OP = mybir.AluOpType

    nc = bacc.Bacc()

    # ---- DRAM parameters ----
    d_xt = nc.declare_dram_parameter("xt_full", [T, BS], f32, isOutput=False)
    d_epsT = nc.declare_dram_parameter("epsT", [H, BSH], f32, isOutput=False)
    d_whhT = nc.declare_dram_parameter("whhT", [H, 3 * H], f32, isOutput=False)
    d_w1 = nc.declare_dram_parameter("w1", [T, 100], f32, isOutput=False)
    d_w2 = nc.declare_dram_parameter("w2", [100, 100], f32, isOutput=False)
    d_w3 = nc.declare_dram_parameter("w3", [100, 2 * H], f32, isOutput=False)
    d_prow = nc.declare_dram_parameter("prow", [2, NROW], f32, isOutput=False)
    d_pcol = nc.declare_dram_parameter("pcol", [128, NCOL], f32, isOutput=False)
    d_ident = nc.declare_dram_parameter("ident", [128, 128], f32, isOutput=False)
    f16 = mybir.dt.float16
    d_prow16 = nc.declare_dram_parameter("prow16", [1, BSH * T + 128], f16, isOutput=False)
    d_pcol16 = nc.declare_dram_parameter("pcol16", [128, 2 * HC], f16, isOutput=False)
    d_iw16 = nc.declare_dram_parameter("iw16", [2, 2 + 4 * T], f16, isOutput=False)
    d_pred = nc.declare_dram_parameter("out_pred", [BSH // 2, 4 * T], f16, isOutput=True)
    d_z = nc.declare_dram_parameter("out_z", [BSH, H], f32, isOutput=True)
    d_loss = nc.declare_dram_parameter("out_loss", [1, 1], f32, isOutput=True)

    with tile.TileContext(nc) as tc, ExitStack() as ctx:
        const = ctx.enter_context(tc.tile_pool(name="const", bufs=1))

        # ---- load persistent tensors ----
        prow = const.tile([2, NROW], f32, name="prow", tag="prow")
        nc.sync.dma_start(prow[:], d_prow[:])
        pcol = const.tile([128, NCOL], f32, name="pcol", tag="pcol")
        nc.sync.dma_start(pcol[:], d_pcol[:])
        prow16 = const.tile([1, BSH * T + 128], f16, name="prow16", tag="prow16")
        nc.sync.dma_start(prow16[:], d_prow16[:])
        pcol16 = const.tile([128, 2 * HC], f16, name="pcol16", tag="pcol16")
        nc.sync.dma_start(pcol16[:], d_pcol16[:])
        iw16 = const.tile([2, 2 + 4 * T], f16, name="iw16", tag="iw16")
        nc.sync.dma_start(iw16[:], d_iw16[:])
        zT = [const.tile([128, BSH], f32, name=f"zT{c}", tag=f"zT{c}") for c in range(HC)]
        ghT = [const.tile([128, BSH], f32, name=f"ghT{g}", tag=f"ghT{g}")
               for g in range(3 * HC)]  # feature-major gh columns, r/z/n x 4 chunks

        with tc.tile_pool(name="epsum", bufs=1, space="PSUM") as epsum, \
             tc.tile_pool(name="esb", bufs=2) as esb, \
             tc.tile_pool(name="enc", bufs=1) as enc, \
             tc.tile_pool(name="stat", bufs=1) as stat:
            ident = enc.tile([128, 128], f32, name="ident", tag="ident")
            nc.sync.dma_start(ident[:], d_ident[:])
            xt = []
            for k in range(2):
                t_ = enc.tile([128, BS], f32, name=f"xt{k}", tag=f"xt{k}")
                nc.sync.dma_start(t_[:], d_xt[128 * k:128 * (k + 1), :])
                xt.append(t_)
            w1t = []
            for k in range(2):
                t_ = enc.tile([128, 100], f32, name=f"w1t{k}", tag=f"w1t{k}")
                nc.sync.dma_start(t_[:], d_w1[128 * k:128 * (k + 1), :])
                w1t.append(t_)
            w2t = enc.tile([100, 100], f32, name="w2t", tag="w2t")
            nc.sync.dma_start(w2t[:], d_w2[:])
            w3t = enc.tile([100, 2 * H], f32, name="w3t", tag="w3t")
            nc.sync.dma_start(w3t[:], d_w3[:])
            whhT = []
            for k in range(HC):
                t_ = enc.tile([128, 3 * H], f32, name=f"whhT{k}", tag=f"whhT{k}")
                nc.sync.dma_start(t_[:], d_whhT[128 * k:128 * (k + 1), :])
                whhT.append(t_)
            epsT = []
            for c in range(HC):
                t_ = enc.tile([128, BSH], f32, name=f"epsT{c}", tag=f"epsT{c}")
                nc.sync.dma_start(t_[:], d_epsT[128 * c:128 * (c + 1), :])
                epsT.append(t_)
            mu = [enc.tile([128, BS], f32, name=f"mu{c}", tag=f"mu{c}") for c in range(HC)]
            std = [enc.tile([128, BS], f32, name=f"std{c}", tag=f"std{c}") for c in range(HC)]
            gh_bm = enc.tile([BSH, 3 * H], f32, name="gh_bm", tag="gh_bm")

            def batchnorm_layer(psum_in, bias_col, g_col, bb_col, nparts):
                """relu(psum+bias) -> batchnorm -> normalized tile"""
                h_ = esb.tile([nparts, BS], f32, name="h_", tag="h_")
                s_sum = stat.tile([nparts, 1], f32, name="s_sum", tag="s_sum")
                nc.scalar.activation(h_[:], psum_in, AF.Relu, bias=bias_col,
                                     accum_out=s_sum[:])
                sq = esb.tile([nparts, BS], f32, name="sq", tag="sq")
                s_sq = stat.tile([nparts, 1], f32, name="s_sq", tag="s_sq")
                nc.scalar.activation(sq[:], h_[:], AF.Square, accum_out=s_sq[:])
                m = stat.tile([nparts, 1], f32, name="m", tag="m")
                nc.vector.tensor_scalar_mul(m[:], s_sum[:], 1.0 / BS)
                v = stat.tile([nparts, 1], f32, name="v", tag="v")
                nc.vector.tensor_scalar_mul(v[:], s_sq[:], 1.0 / BS)
                m2 = stat.tile([nparts, 1], f32, name="m2", tag="m2")
                nc.vector.tensor_tensor(m2[:], m[:], m[:], OP.mult)
                var = stat.tile([nparts, 1], f32, name="var", tag="var")
                nc.vector.tensor_tensor(var[:], v[:], m2[:], OP.subtract)
                sd = stat.tile([nparts, 1], f32, name="sd", tag="sd")
                nc.scalar.activation(sd[:], var[:], AF.Sqrt, bias=pcol[:nparts, C_EPS:C_EPS + 1])
                rs = stat.tile([nparts, 1], f32, name="rs", tag="rs")
                nc.vector.reciprocal(rs[:], sd[:])
                sc = stat.tile([nparts, 1], f32, name="sc", tag="sc")
                nc.vector.tensor_tensor(sc[:], rs[:], g_col, OP.mult)
                msc = stat.tile([nparts, 1], f32, name="msc", tag="msc")
                nc.vector.tensor_tensor(msc[:], m[:], sc[:], OP.mult)
                sh = stat.tile([nparts, 1], f32, name="sh", tag="sh")
                nc.vector.tensor_tensor(sh[:], bb_col, msc[:], OP.subtract)
                hn = esb.tile([nparts, BS], f32, name="hn", tag="hn")
                nc.scalar.activation(hn[:], h_[:], AF.Identity, bias=sh[:],
                                     scale=sc[:])
                return hn

            # ---- encoder layer 1 ----
            ps1 = epsum.tile([100, BS], f32, name="ps1", tag="ps1")
            nc.tensor.matmul(ps1[:], w1t[0][:], xt[0][:], start=True, stop=False)
            nc.tensor.matmul(ps1[:], w1t[1][:], xt[1][:], start=False, stop=True)
            h1n = batchnorm_layer(ps1[:], pcol[:100, C_B1:C_B1 + 1],
                                  pcol[:100, C_G1:C_G1 + 1],
                                  pcol[:100, C_BB1:C_BB1 + 1], 100)
            # ---- encoder layer 2 ----
            ps2 = epsum.tile([100, BS], f32, name="ps2", tag="ps2")
            nc.tensor.matmul(ps2[:], w2t[:], h1n[:], start=True, stop=True)
            h2n = batchnorm_layer(ps2[:], pcol[:100, C_B2:C_B2 + 1],
                                  pcol[:100, C_G2:C_G2 + 1],
                                  pcol[:100, C_BB2:C_BB2 + 1], 100)

            # ---- encoder layer 3: mu / logvar(std), loss stats ----
            lsums = []
            for c in range(HC):
                ps3 = epsum.tile([128, BS], f32, name="ps3", tag="ps3")
                nc.tensor.matmul(ps3[:], w3t[:, 128 * c:128 * (c + 1)],
                                 h2n[:], start=True, stop=True)
                nc.scalar.activation(mu[c][:], ps3[:], AF.Identity,
                                     bias=pcol[:, C_B3MU + c:C_B3MU + c + 1])
                smu2 = stat.tile([128, 1], f32, name=f"smu2{c}", tag=f"smu2{c}")
                sq2 = esb.tile([128, BS], f32, name="sq2", tag="sq2")
                nc.scalar.activation(sq2[:], mu[c][:], AF.Square, accum_out=smu2[:])

                ps3b = epsum.tile([128, BS], f32, name="ps3b", tag="ps3b")
                nc.tensor.matmul(ps3b[:], w3t[:, H + 128 * c:H + 128 * (c + 1)],
                                 h2n[:], start=True, stop=True)
                nc.scalar.activation(std[c][:], ps3b[:], AF.Exp, scale=0.5,
                                     bias=pcol[:, C_HB3LV + c:C_HB3LV + c + 1])
                slv = stat.tile([128, 1], f32, name=f"slv{c}", tag=f"slv{c}")
                lvt = esb.tile([128, BS], f32, name="lvt", tag="lvt")
                nc.scalar.activation(lvt[:], ps3b[:], AF.Identity,
                                     bias=pcol[:, C_B3LV + c:C_B3LV + c + 1],
                                     accum_out=slv[:])
                sstd2 = stat.tile([128, 1], f32, name=f"sstd2{c}", tag=f"sstd2{c}")
                sq3 = esb.tile([128, BS], f32, name="sq3", tag="sq3")
                nc.scalar.activation(sq3[:], std[c][:], AF.Square, accum_out=sstd2[:])

                lt = stat.tile([128, 1], f32, name=f"lt{c}", tag=f"lt{c}")
                nc.vector.tensor_tensor(lt[:], slv[:], smu2[:], OP.subtract)
                nc.vector.tensor_tensor(lt[:], lt[:], sstd2[:], OP.subtract)
                lsums.append(lt)

                # z (feature-major) for this core's shard = columns 0:64
                tmp = stat.tile([128, BSH], f32, name="ztmp", tag="ztmp")
                nc.vector.tensor_tensor(tmp[:], epsT[c][:], std[c][:, :BSH], OP.mult)
                nc.vector.tensor_tensor(zT[c][:], tmp[:], mu[c][:, :BSH], OP.add)

            # ---- latent loss ----
            stot = stat.tile([128, 1], f32, name="stot", tag="stot")
            nc.vector.tensor_tensor(stot[:], lsums[0][:], lsums[1][:], OP.add)
            nc.vector.tensor_tensor(stot[:], stot[:], lsums[2][:], OP.add)
            nc.vector.tensor_tensor(stot[:], stot[:], lsums[3][:], OP.add)
            psl = epsum.tile([1, 1], f32, name="psl", tag="psl")
            nc.tensor.matmul(psl[:], stot[:], pcol[:, C_ONES:C_ONES + 1],
                             start=True, stop=True)
            lossv = stat.tile([1, 1], f32, name="lossv", tag="lossv")
            nc.scalar.activation(lossv[:], psl[:], AF.Identity,
                                 scale=-1.0 / (2.0 * BS),
                                 bias=pcol[0:1, C_N256:C_N256 + 1])
            nc.sync.dma_start(d_loss[:], lossv[:])

            # ---- z output: transpose zT chunks -> batch-major, DMA out ----
            zbm = enc.tile([BSH, H], f32, name="zbm", tag="zbm")
            for c in range(HC):
                pzt = epsum.tile([BSH, 128], f32, name="pzt", tag="pzt")
                nc.tensor.transpose(pzt[:], zT[c][:], ident[:])
                nc.vector.tensor_copy(zbm[:, 128 * c:128 * (c + 1)], pzt[:])
            nc.sync.dma_start(d_z[:], zbm[:])

            # ---- gh = z @ w_hh.T + bsum (batch-major, then feature-major cols) ----
            ones64 = prow[0:1, ONES_OFF:ONES_OFF + BSH]  # ones row, partition 0
            for mg in range(3):
                sl = slice(H * mg, H * (mg + 1))
                pg = epsum.tile([BSH, H], f32, name="pg", tag="pg")
                for k in range(HC):
                    nc.tensor.matmul(pg[:], zT[k][:], whhT[k][:, sl],
                                     start=(k == 0), stop=False)
                nc.tensor.matmul(pg[:], ones64,
                                 prow[0:1, BSUM_OFF + H * mg:BSUM_OFF + H * (mg + 1)],
                                 start=False, stop=True)
                nc.vector.tensor_copy(gh_bm[:, sl], pg[:])
            for g in range(3 * HC):
                pt = epsum.tile([128, BSH], f32, name="pt", tag="pt")
                nc.tensor.transpose(pt[:], gh_bm[:, 128 * g:128 * (g + 1)],
                                    ident[:BSH, :BSH])
                nc.vector.tensor_copy(ghT[g][:], pt[:])

        # ---- decode loop (fp16, quads of 4 batches) ----
        QG = 4  # batches per group
        NG = BSH // QG
        with tc.tile_pool(name="xpsum", bufs=2, space="PSUM") as xpsum, \
             tc.tile_pool(name="opsum", bufs=2, space="PSUM") as opsum, \
             tc.tile_pool(name="dec", bufs=4) as dec:
            ones16 = prow16[0:1, BSH * T:BSH * T + 128]
            FD = QG * T  # 1024
            for g in range(NG):
                bs = [QG * g + i for i in range(QG)]
                b0 = bs[0]
                ps_x = xpsum.tile([128, FD], f32, name="ps_x", tag="ps_x")
                for h in range(2):
                    nc.tensor.matmul(ps_x[:, 512 * h:512 * (h + 1)], ones16,
                                     prow16[0:1, T * b0 + 512 * h:T * b0 + 512 * (h + 1)],
                                     start=True, stop=True)
                xsb = dec.tile([128, FD], f16, name="xsb", tag="xsb")
                nc.vector.tensor_copy(xsb[:], ps_x[:])
                qr_tiles = []
                for c in range(HC):
                    gin = dec.tile([128, FD], f16, name="gin", tag="gin")
                    nc.gpsimd.tensor_scalar(gin[:], xsb[:],
                                            pcol[:, C_WN + c:C_WN + c + 1],
                                            pcol[:, C_BIHN + c:C_BIHN + c + 1],
                                            OP.mult, OP.add)
                    r = dec.tile([128, FD], f16, name="r", tag="r")
                    zt_ = dec.tile([128, FD], f16, name="zt_", tag="zt_")
                    for i, b in enumerate(bs):
                        ts_ = slice(T * i, T * (i + 1))
                        nc.scalar.activation(r[:, ts_], xsb[:, ts_], AF.Sigmoid,
                                             scale=pcol[:, C_WR + c:C_WR + c + 1],
                                             bias=ghT[c][:, b:b + 1])
                        nc.scalar.activation(zt_[:, ts_], xsb[:, ts_], AF.Sigmoid,
                                             scale=pcol[:, C_WZ + c:C_WZ + c + 1],
                                             bias=ghT[HC + c][:, b:b + 1])
                    t1 = dec.tile([128, FD], f16, name="t1", tag="t1")
                    for i, b in enumerate(bs):
                        ts_ = slice(T * i, T * (i + 1))
                        nc.vector.scalar_tensor_tensor(
                            t1[:, ts_], r[:, ts_], ghT[2 * HC + c][:, b:b + 1],
                            gin[:, ts_], OP.mult, OP.add)
                    n_ = dec.tile([128, FD], f16, name="n_", tag="n_")
                    nc.scalar.activation(n_[:], t1[:], AF.Tanh)
                    pp = dec.tile([128, FD], f16, name="pp", tag="pp")
                    for i, b in enumerate(bs):
                        ts_ = slice(T * i, T * (i + 1))
                        nc.vector.scalar_tensor_tensor(
                            pp[:, ts_], n_[:, ts_], zT[c][:, b:b + 1],
                            zt_[:, ts_], OP.subtract, OP.mult)
                    q = dec.tile([128, FD], f16, name="q", tag="q")
                    nc.vector.tensor_tensor(q[:], n_[:], pp[:], OP.subtract)
                    qr = dec.tile([128, FD], f16, name="qr", tag="qr")
                    nc.vector.tensor_scalar_max(qr[:], q[:], 0.0)
                    qr_tiles.append(qr)
                ps_o = opsum.tile([2, FD], f32, name="ps_o", tag="ps_o")
                for h in range(2):
                    nc.tensor.matmul(ps_o[:, 512 * h:512 * (h + 1)],
                                     iw16[:, 0:2],
                                     iw16[:, 2 + 512 * h:2 + 512 * (h + 1)],
                                     start=True, stop=False)
                for c in range(HC):
                    for h in range(2):
                        nc.tensor.matmul(ps_o[:, 512 * h:512 * (h + 1)],
                                         pcol16[:, 2 * c:2 * c + 2],
                                         qr_tiles[c][:, 512 * h:512 * (h + 1)],
                                         start=False, stop=(c == HC - 1))
                po = dec.tile([2, FD], f16, name="po", tag="po")
                nc.vector.tensor_copy(po[:], ps_o[:])
                nc.sync.dma_start(d_pred[2 * g:2 * g + 2, :], po[:])

    nc.finalize()
    return nc


def _host_prep(x, eps, enc_w1, enc_b1, bn1_g, bn1_b, enc_w2, enc_b2, bn2_g, bn2_b,
               enc_w3, enc_b3, gru_w_ih, gru_w_hh, gru_b_ih, gru_b_hh, fc_w, fc_b,
               k_binomial, k_activity):
    """Build per-core input maps (all numpy, f32)."""
    f = np.float32
    x = np.asarray(x, f)
    xs = x[:, 0, :T]                      # [512, 256] (also the encoder input)
    xt_base = np.ascontiguousarray(xs.T)  # [256, 512]
    whhT = np.ascontiguousarray(np.asarray(gru_w_hh, f).T)  # [512, 1536]
    w_ih = np.asarray(gru_w_ih, f)[:, 0]  # [1536]
    b_ih = np.asarray(gru_b_ih, f)
    b_hh = np.asarray(gru_b_hh, f)
    bsum = np.concatenate([(b_ih + b_hh)[:2 * H], b_hh[2 * H:]]).astype(f)

    # AddBeta weights + fc_b
    idx = np.arange(T)
    base = np.where(idx % 2 == 0, 0.5, 0.2).astype(f)
    l = T_CROSS / 5.0
    j = (idx // 4).astype(f)
    wb = np.exp(-((j - (T_CROSS - 1)) ** 2) / (2 * l * l))
    wa = np.exp(-(j ** 2) / (2 * l * l))
    off = base * np.where(idx % 4 < 2, wb, wa)
    eff = np.where(idx >= T_CROSS, -1.0, 1.0).astype(f)
    k = np.stack([np.asarray(k_binomial, f)[0], np.asarray(k_activity, f)[0]])
    wpb = (eff[None, :] * (np.logaddexp(0.0, k) + off[None, :])
           + np.asarray(fc_b, f)[:, None]).astype(f)        # [2, 256]

    pcol = np.zeros((128, NCOL), f)
    b3 = np.asarray(enc_b3, f)
    pcol[:100, C_B1] = np.asarray(enc_b1, f)
    pcol[:100, C_G1] = np.asarray(bn1_g, f)
    pcol[:100, C_BB1] = np.asarray(bn1_b, f)
    pcol[:100, C_B2] = np.asarray(enc_b2, f)
    pcol[:100, C_G2] = np.asarray(bn2_g, f)
    pcol[:100, C_BB2] = np.asarray(bn2_b, f)
    for c in range(HC):
        pcol[:, C_B3MU + c] = b3[128 * c:128 * (c + 1)]
        pcol[:, C_HB3LV + c] = 0.5 * b3[H + 128 * c:H + 128 * (c + 1)]
        pcol[:, C_B3LV + c] = b3[H + 128 * c:H + 128 * (c + 1)]
        pcol[:, C_FCW + 2 * c] = np.asarray(fc_w, f)[128 * c:128 * (c + 1), 0]
        pcol[:, C_FCW + 2 * c + 1] = np.asarray(fc_w, f)[128 * c:128 * (c + 1), 1]
    pcol[:, C_ONES] = 1.0
    pcol[:, C_EPS] = BN_EPS
    pcol[:, C_N256] = -float(H) / 2.0
    for c in range(HC):
        pcol[:, C_WR + c] = w_ih[128 * c:128 * (c + 1)]
        pcol[:, C_WZ + c] = w_ih[H + 128 * c:H + 128 * (c + 1)]
        pcol[:, C_WN + c] = w_ih[2 * H + 128 * c:2 * H + 128 * (c + 1)]
        pcol[:, C_BIHN + c] = b_ih[2 * H + 128 * c:2 * H + 128 * (c + 1)]
    pcol16 = np.zeros((128, 2 * HC), np.float16)
    for c in range(HC):
        pcol16[:, 2 * c] = np.asarray(fc_w, f)[128 * c:128 * (c + 1), 0]
        pcol16[:, 2 * c + 1] = np.asarray(fc_w, f)[128 * c:128 * (c + 1), 1]

    ident = np.eye(128, dtype=f)
    eps = np.asarray(eps, f)

    in_maps = []
    for ci in range(NCORES):
        b0 = ci * BSH
        prow = np.zeros((2, NROW), f)
        for rep in range(4):
            prow[0, WPB_OFF + T * rep:WPB_OFF + T * (rep + 1)] = wpb[0]
            prow[1, WPB_OFF + T * rep:WPB_OFF + T * (rep + 1)] = wpb[1]
        prow[0, BSUM_OFF:BSUM_OFF + 3 * H] = bsum
        prow[0, ONES_OFF:ONES_OFF + BSH] = 1.0
        iw16 = np.zeros((2, 2 + 4 * T), np.float16)
        iw16[0, 0] = 1.0
        iw16[1, 1] = 1.0
        for rep in range(4):
            iw16[0, 2 + T * rep:2 + T * (rep + 1)] = wpb[0]
            iw16[1, 2 + T * rep:2 + T * (rep + 1)] = wpb[1]
        in_maps.append({
            "iw16": iw16,
            "prow16": np.concatenate([xs[b0:b0 + BSH].reshape(-1), np.ones(128, f)]).reshape(1, -1).astype(np.float16),
            "pcol16": pcol16,
            "xt_full": np.ascontiguousarray(np.roll(xt_base, -b0, axis=1)),
            "epsT": np.ascontiguousarray(eps[b0:b0 + BSH].T),
            "whhT": whhT,
            "w1": np.asarray(enc_w1, f),
            "w2": np.asarray(enc_w2, f),
            "w3": np.asarray(enc_w3, f),
            "prow": prow,
            "pcol": pcol,
            "ident": ident,
        })
    return in_maps


_NC_CACHE = None
TRACE = False
LAST_RESULT = None


def kernel(**inputs):
    global _NC_CACHE, LAST_RESULT
    from concourse.bass_utils import run_bass_kernel_spmd

    in_maps = _host_prep(**inputs)
    if _NC_CACHE is None:
        _NC_CACHE = _build_program()
    nc = _NC_CACHE
    res = run_bass_kernel_spmd(nc, in_maps, list(range(NCORES)), trace=TRACE)
    LAST_RESULT = res
    results = res.results

    predict = np.zeros((BS, 1, 2, T), np.float32)
    z = np.zeros((BS, H), np.float32)
    for ci in range(NCORES):
        b0 = ci * BSH
        pred = np.asarray(results[ci]["out_pred"]).astype(np.float32)  # [32, 1024]
        # rows (2g, 2g+1) hold [c, (b0..b3 t)] for the quad
        pred = pred.reshape(BSH // 4, 2, 4, T).transpose(0, 2, 1, 3).reshape(BSH, 2, T)
        predict[b0:b0 + BSH, 0] = pred
        z[b0:b0 + BSH] = np.asarray(results[ci]["out_z"])
    loss = np.float32(np.asarray(results[0]["out_loss"]).reshape(-1)[0])
    return (predict, loss, z)


# revision 40
# speedup vs baseline: 1.0093x; 1.0093x over previous
"""Trainium2 Bass kernel for nn_AddBeta (VAE encoder + parallel-GRU decode + AddBeta).

Sharding: pure data parallel over batch. Each of the 8 cores gets a 64-batch
shard; the tiny encoder MLP (whose BatchNorm uses full-batch statistics) is
computed replicated on every core with the batch ROTATED per core so that each
core's own 64 batches sit in columns 0:64 of the feature-major activations
(BN stats and the latent loss are batch-permutation invariant). No collectives.

Encoder (fp32): x^T chunks stream through 3 matmuls; BN stats come free via
activation accum_out; mu/std/logvar chunks feed z = mu + eps*std (feature-
major), the latent loss (ones-matmul partition reduce), and gh = z @ w_hh.T +
biases (batch-major via PE, then 12 PE transposes give per-batch gh columns).

Decode (fp16, batch-quads of 4): xs is broadcast across partitions once per
quad by a PE outer product (ones x xs -> PSUM -> fp16 SBUF tile). Gate
pre-activations then need NO matmuls: r/z come from ScalarE activations with
per-feature weight as `scale` and per-batch gh column as `bias`; the n-gate
input is a GpSimd tensor_scalar. The GRU combine is fused into per-batch
scalar_tensor_tensor ops on VectorE, tanh/sigmoid on ScalarE (both LUT funcs
share one act table - no table thrash). The output head accumulates 4 feature
chunks in PSUM on top of the AddBeta weights preloaded by an I2 x wpb fp16
matmul; results DMA out as fp16 and are upcast on host.

Engine balance per quad-chunk: ScalarE 8 sigmoid + 1 tanh, VectorE 8 fused
stt + q + relu + evacs, GpSimd gin, PE 2 bcast + 8+2 head matmuls.
"""

import numpy as np

# ---- problem constants (hardcoded; kernel.py must be self-contained) ----
BS = 512          # total batch
NCORES = 8
BSH = BS // NCORES  # 64 batches per core
T = 256           # output length
H = 512           # latent
HC = H // 128     # 4 feature chunks
PAIRS = BSH // 2  # 32
BN_EPS = 1e-5
T_CROSS = 20

# prow packed-row offsets (free dim of a [2, NROW] f32 tensor)
WPB_OFF = 0                 # [2,1024]: AddBeta weights + fc_b, tiled x4
BSUM_OFF = WPB_OFF + 4 * T  # [1,1536]: b_ih+b_hh (r,z) / b_hh (n) in row0
ONES_OFF = BSUM_OFF + 3 * H  # [1,64]: ones in row0
NROW = ONES_OFF + BSH

# pcol packed-column indices ([128, NCOL] f32)
C_B1, C_G1, C_BB1, C_B2, C_G2, C_BB2 = 0, 1, 2, 3, 4, 5
C_B3MU = 6    # 6..9
C_HB3LV = 10  # 10..13
C_B3LV = 14   # 14..17
C_ONES = 18
C_FCW = 19    # 19+2c, 20+2c for chunk c
C_EPS = 27
C_N256 = 28
C_WR = 29     # 29..32: w_ih r-gate chunk cols
C_WZ = 33     # 33..36: z-gate
C_WN = 37     # 37..40: n-gate
C_BIHN = 41   # 41..44: b_ih n-gate
NCOL = 45


def _build_program():
    import concourse.bass as bass
    import concourse.bacc as bacc
    import concourse.tile as tile
    from concourse import mybir
    from contextlib import ExitStack

    f32 = mybir.dt.float32
    f32r = mybir.dt.float32r
    AF = mybir.ActivationFunctionType
    OP = mybir.AluOpType

    def rmm(out, lhsT, rhs, **kw):
        nc.tensor.matmul(out, lhsT, rhs, **kw)

    nc = bacc.Bacc()

    # ---- DRAM parameters ----
    d_xt = nc.declare_dram_parameter("xt_full", [T, BS], f32, isOutput=False)
    d_epsT = nc.declare_dram_parameter("epsT", [H, BSH], f32, isOutput=False)
    d_whhT = nc.declare_dram_parameter("whhT", [H, 3 * H], f32, isOutput=False)
    d_w1 = nc.declare_dram_parameter("w1", [T, 100], f32, isOutput=False)
    d_w2 = nc.declare_dram_parameter("w2", [100, 100], f32, isOutput=False)
    d_w3 = nc.declare_dram_parameter("w3", [100, 2 * H], f32, isOutput=False)
    d_prow = nc.declare_dram_parameter("prow", [2, NROW], f32, isOutput=False)
    d_pcol = nc.declare_dram_parameter("pcol", [128, NCOL], f32, isOutput=False)
    d_ident = nc.declare_dram_parameter("ident", [128, 128], f32, isOutput=False)
    f16 = mybir.dt.float16
    d_prow16 = nc.declare_dram_parameter("prow16", [1, BSH * T + 128], f16, isOutput=False)
    d_pcol16 = nc.declare_dram_parameter("pcol16", [128, 2 * HC], f16, isOutput=False)
    d_iw16 = nc.declare_dram_parameter("iw16", [2, 2 + 4 * T], f16, isOutput=False)
    d_pred = nc.declare_dram_parameter("out_pred", [BSH // 2, 4 * T], f16, isOutput=True)
    d_z = nc.declare_dram_parameter("out_z", [BSH, H], f32, isOutput=True)
    d_loss = nc.declare_dram_parameter("out_loss", [1, 1], f32, isOutput=True)

    with tile.TileContext(nc) as tc, ExitStack() as ctx:
        const = ctx.enter_context(tc.tile_pool(name="const", bufs=1))

        # ---- load persistent tensors ----
        prow = const.tile([2, NROW], f32, name="prow", tag="prow")
        nc.sync.dma_start(prow[:], d_prow[:])
        pcol = const.tile([128, NCOL], f32, name="pcol", tag="pcol")
        nc.sync.dma_start(pcol[:], d_pcol[:])
        prow16 = const.tile([1, BSH * T + 128], f16, name="prow16", tag="prow16")
        nc.sync.dma_start(prow16[:], d_prow16[:])
        pcol16 = const.tile([128, 2 * HC], f16, name="pcol16", tag="pcol16")
        nc.sync.dma_start(pcol16[:], d_pcol16[:])
        iw16 = const.tile([2, 2 + 4 * T], f16, name="iw16", tag="iw16")
        nc.sync.dma_start(iw16[:], d_iw16[:])
        zT = [const.tile([128, BSH], f32, name=f"zT{c}", tag=f"zT{c}") for c in range(HC)]
        ghT = [const.tile([128, BSH], f32, name=f"ghT{g}", tag=f"ghT{g}")
               for g in range(3 * HC)]  # feature-major gh columns, r/z/n x 4 chunks

        late = ctx.enter_context(tc.tile_pool(name="late", bufs=1))
        with tc.tile_pool(name="epsum", bufs=1, space="PSUM") as epsum, \
             tc.tile_pool(name="esb", bufs=2) as esb, \
             tc.tile_pool(name="enc", bufs=1) as enc, \
             tc.tile_pool(name="stat", bufs=1) as stat:
            ident = late.tile([128, 128], f32, name="ident", tag="ident")
            nc.sync.dma_start(ident[:], d_ident[:])
            xt = []
            for k in range(2):
                t_ = enc.tile([128, BS], f32, name=f"xt{k}", tag=f"xt{k}")
                nc.sync.dma_start(t_[:], d_xt[128 * k:128 * (k + 1), :])
                xt.append(t_)
            w1t = []
            for k in range(2):
                t_ = enc.tile([128, 100], f32, name=f"w1t{k}", tag=f"w1t{k}")
                nc.sync.dma_start(t_[:], d_w1[128 * k:128 * (k + 1), :])
                w1t.append(t_)
            w2t = enc.tile([100, 100], f32, name="w2t", tag="w2t")
            nc.sync.dma_start(w2t[:], d_w2[:])
            w3t = enc.tile([100, 2 * H], f32, name="w3t", tag="w3t")
            nc.sync.dma_start(w3t[:], d_w3[:])
            whhT = []
            for k in range(HC):
                t_ = enc.tile([128, 3 * H], f32, name=f"whhT{k}", tag=f"whhT{k}")
                nc.sync.dma_start(t_[:], d_whhT[128 * k:128 * (k + 1), :])
                whhT.append(t_)
            epsT = []
            for c in range(HC):
                t_ = enc.tile([128, BSH], f32, name=f"epsT{c}", tag=f"epsT{c}")
                nc.sync.dma_start(t_[:], d_epsT[128 * c:128 * (c + 1), :])
                epsT.append(t_)
            mu = [late.tile([128, BS], f32, name=f"mu{c}", tag=f"mu{c}") for c in range(HC)]
            std = [late.tile([128, BS], f32, name=f"std{c}", tag=f"std{c}") for c in range(HC)]
            lv = [late.tile([128, BS], f32, name=f"lv{c}", tag=f"lv{c}") for c in range(HC)]
            gh_bm = enc.tile([BSH, 3 * H], f32, name="gh_bm", tag="gh_bm")

            def batchnorm_layer(psum_in, bias_col, g_col, bb_col, nparts):
                """relu(psum+bias) -> batchnorm -> normalized tile"""
                h_ = esb.tile([nparts, BS], f32, name="h_", tag="h_")
                s_sum = stat.tile([nparts, 1], f32, name="s_sum", tag="s_sum")
                nc.scalar.activation(h_[:], psum_in, AF.Relu, bias=bias_col,
                                     accum_out=s_sum[:])
                sq = esb.tile([nparts, BS], f32, name="sq", tag="sq")
                s_sq = stat.tile([nparts, 1], f32, name="s_sq", tag="s_sq")
                nc.scalar.activation(sq[:], h_[:], AF.Square, accum_out=s_sq[:])
                m = stat.tile([nparts, 1], f32, name="m", tag="m")
                nc.vector.tensor_scalar_mul(m[:], s_sum[:], 1.0 / BS)
                v = stat.tile([nparts, 1], f32, name="v", tag="v")
                nc.vector.tensor_scalar_mul(v[:], s_sq[:], 1.0 / BS)
                m2 = stat.tile([nparts, 1], f32, name="m2", tag="m2")
                nc.vector.tensor_tensor(m2[:], m[:], m[:], OP.mult)
                var = stat.tile([nparts, 1], f32, name="var", tag="var")
                nc.vector.tensor_tensor(var[:], v[:], m2[:], OP.subtract)
                sd = stat.tile([nparts, 1], f32, name="sd", tag="sd")
                nc.scalar.activation(sd[:], var[:], AF.Sqrt, bias=pcol[:nparts, C_EPS:C_EPS + 1])
                rs = stat.tile([nparts, 1], f32, name="rs", tag="rs")
                nc.vector.reciprocal(rs[:], sd[:])
                sc = stat.tile([nparts, 1], f32, name="sc", tag="sc")
                nc.vector.tensor_tensor(sc[:], rs[:], g_col, OP.mult)
                msc = stat.tile([nparts, 1], f32, name="msc", tag="msc")
                nc.vector.tensor_tensor(msc[:], m[:], sc[:], OP.mult)
                sh = stat.tile([nparts, 1], f32, name="sh", tag="sh")
                nc.vector.tensor_tensor(sh[:], bb_col, msc[:], OP.subtract)
                hn = esb.tile([nparts, BS], f32, name="hn", tag="hn")
                nc.scalar.activation(hn[:], h_[:], AF.Identity, bias=sh[:],
                                     scale=sc[:])
                return hn

            # ---- encoder layer 1 ----
            ps1 = epsum.tile([100, BS], f32, name="ps1", tag="ps1")
            rmm(ps1[:], w1t[0][:], xt[0][:], start=True, stop=False)
            rmm(ps1[:], w1t[1][:], xt[1][:], start=False, stop=True)
            h1n = batchnorm_layer(ps1[:], pcol[:100, C_B1:C_B1 + 1],
                                  pcol[:100, C_G1:C_G1 + 1],
                                  pcol[:100, C_BB1:C_BB1 + 1], 100)
            # ---- encoder layer 2 ----
            ps2 = epsum.tile([100, BS], f32, name="ps2", tag="ps2")
            rmm(ps2[:], w2t[:], h1n[:], start=True, stop=True)
            h2n = batchnorm_layer(ps2[:], pcol[:100, C_B2:C_B2 + 1],
                                  pcol[:100, C_G2:C_G2 + 1],
                                  pcol[:100, C_BB2:C_BB2 + 1], 100)

            # ---- encoder layer 3: mu / logvar(std), loss stats ----
            for c in range(HC):
                ps3 = epsum.tile([128, BS], f32, name="ps3", tag="ps3")
                rmm(ps3[:], w3t[:, 128 * c:128 * (c + 1)],
                    h2n[:], start=True, stop=True)
                nc.scalar.activation(mu[c][:], ps3[:], AF.Identity,
                                     bias=pcol[:, C_B3MU + c:C_B3MU + c + 1])
                ps3b = epsum.tile([128, BS], f32, name="ps3b", tag="ps3b")
                rmm(ps3b[:], w3t[:, H + 128 * c:H + 128 * (c + 1)],
                    h2n[:], start=True, stop=True)
                nc.scalar.activation(std[c][:], ps3b[:], AF.Exp, scale=0.5,
                                     bias=pcol[:, C_HB3LV + c:C_HB3LV + c + 1])
                nc.scalar.activation(lv[c][:], ps3b[:], AF.Identity,
                                     bias=pcol[:, C_B3LV + c:C_B3LV + c + 1])

                # z (feature-major) for this core's shard = columns 0:64
                tmp = stat.tile([128, BSH], f32, name="ztmp", tag="ztmp")
                nc.vector.tensor_tensor(tmp[:], epsT[c][:], std[c][:, :BSH], OP.mult)
                nc.vector.tensor_tensor(zT[c][:], tmp[:], mu[c][:, :BSH], OP.add)

            # ---- gh = z @ w_hh.T + bsum (batch-major, then feature-major cols) ----
            ones64 = prow[0:1, ONES_OFF:ONES_OFF + BSH]  # ones row, partition 0
            for mg in range(3):
                sl = slice(H * mg, H * (mg + 1))
                pg = epsum.tile([BSH, H], f32, name="pg", tag="pg", bufs=3)
                for k in range(HC):
                    rmm(pg[:], zT[k][:], whhT[k][:, sl],
                        start=(k == 0), stop=False)
                rmm(pg[:], ones64,
                    prow[0:1, BSUM_OFF + H * mg:BSUM_OFF + H * (mg + 1)],
                    start=False, stop=True)
                nc.vector.tensor_copy(gh_bm[:, sl], pg[:])
            for g in range(3 * HC):
                pt = epsum.tile([128, BSH], f32, name="pt", tag="pt")
                nc.tensor.transpose(pt[:], gh_bm[:, 128 * g:128 * (g + 1)],
                                    ident[:BSH, :BSH])
                nc.vector.tensor_copy(ghT[g][:], pt[:])

        # ---- decode loop (fp16, quads of 4 batches) ----
        QG = 4  # batches per group
        NG = BSH // QG
        with tc.tile_pool(name="xpsum", bufs=2, space="PSUM") as xpsum, \
             tc.tile_pool(name="opsum", bufs=2, space="PSUM") as opsum, \
             tc.tile_pool(name="dec", bufs=4) as dec:
            ones16 = prow16[0:1, BSH * T:BSH * T + 128]
            FD = QG * T  # 1024
            for g in range(NG):
                bs = [QG * g + i for i in range(QG)]
                b0 = bs[0]
                ps_x = xpsum.tile([128, FD], f32, name="ps_x", tag="ps_x")
                for h in range(2):
                    nc.tensor.matmul(ps_x[:, 512 * h:512 * (h + 1)], ones16,
                                     prow16[0:1, T * b0 + 512 * h:T * b0 + 512 * (h + 1)],
                                     start=True, stop=True)
                xsb = dec.tile([128, FD], f16, name="xsb", tag="xsb")
                nc.vector.tensor_copy(xsb[:], ps_x[:])
                qr_tiles = []
                for c in range(HC):
                    gin = dec.tile([128, FD], f16, name="gin", tag="gin")
                    nc.gpsimd.tensor_scalar(gin[:], xsb[:],
                                            pcol[:, C_WN + c:C_WN + c + 1],
                                            pcol[:, C_BIHN + c:C_BIHN + c + 1],
                                            OP.mult, OP.add)
                    r = dec.tile([128, FD], f16, name="r", tag="r")
                    zt_ = dec.tile([128, FD], f16, name="zt_", tag="zt_")
                    for i, b in enumerate(bs):
                        ts_ = slice(T * i, T * (i + 1))
                        nc.scalar.activation(r[:, ts_], xsb[:, ts_], AF.Sigmoid,
                                             scale=pcol[:, C_WR + c:C_WR + c + 1],
                                             bias=ghT[c][:, b:b + 1])
                        nc.scalar.activation(zt_[:, ts_], xsb[:, ts_], AF.Sigmoid,
                                             scale=pcol[:, C_WZ + c:C_WZ + c + 1],
                                             bias=ghT[HC + c][:, b:b + 1])
                    t1 = dec.tile([128, FD], f16, name="t1", tag="t1")
                    for i, b in enumerate(bs):
                        ts_ = slice(T * i, T * (i + 1))
                        nc.vector.scalar_tensor_tensor(
                            t1[:, ts_], r[:, ts_], ghT[2 * HC + c][:, b:b + 1],
                            gin[:, ts_], OP.mult, OP.add)
                    n_ = dec.tile([128, FD], f16, name="n_", tag="n_")
                    nc.scalar.activation(n_[:], t1[:], AF.Tanh)
                    pp = dec.tile([128, FD], f16, name="pp", tag="pp")
                    for i, b in enumerate(bs):
                        ts_ = slice(T * i, T * (i + 1))
                        nc.vector.scalar_tensor_tensor(
                            pp[:, ts_], n_[:, ts_], zT[c][:, b:b + 1],
                            zt_[:, ts_], OP.subtract, OP.mult)
                    q = dec.tile([128, FD], f16, name="q", tag="q")
                    nc.vector.tensor_tensor(q[:], n_[:], pp[:], OP.subtract)
                    qr = dec.tile([128, FD], f16, name="qr", tag="qr")
                    nc.vector.tensor_scalar_max(qr[:], q[:], 0.0)
                    qr_tiles.append(qr)
                ps_o = opsum.tile([2, FD], f32, name="ps_o", tag="ps_o")
                for h in range(2):
                    nc.tensor.matmul(ps_o[:, 512 * h:512 * (h + 1)],
                                     iw16[:, 0:2],
                                     iw16[:, 2 + 512 * h:2 + 512 * (h + 1)],
                                     start=True, stop=False)
                for c in range(HC):
                    for h in range(2):
                        nc.tensor.matmul(ps_o[:, 512 * h:512 * (h + 1)],
                                         pcol16[:, 2 * c:2 * c + 2],
                                         qr_tiles[c][:, 512 * h:512 * (h + 1)],
                                         start=False, stop=(c == HC - 1))
                po = dec.tile([2, FD], f16, name="po", tag="po")
                nc.vector.tensor_copy(po[:], ps_o[:])
                nc.sync.dma_start(d_pred[2 * g:2 * g + 2, :], po[:])

            # ---- deferred: latent loss + z output (fills pipeline slack) ----
            lsums = []
            for c in range(HC):
                smu2 = dec.tile([128, 1], f32, name=f"smu2{c}", tag=f"smu2{c}", bufs=1)
                sq2 = dec.tile([128, BS], f32, name="sq2", tag="sq2", bufs=2)
                nc.scalar.activation(sq2[:], mu[c][:], AF.Square, accum_out=smu2[:])
                sstd2 = dec.tile([128, 1], f32, name=f"sstd2{c}", tag=f"sstd2{c}", bufs=1)
                sq3 = dec.tile([128, BS], f32, name="sq3", tag="sq3", bufs=2)
                nc.scalar.activation(sq3[:], std[c][:], AF.Square, accum_out=sstd2[:])
                slv = dec.tile([128, 1], f32, name=f"slv{c}", tag=f"slv{c}", bufs=1)
                lvt = dec.tile([128, BS], f32, name="lvt", tag="lvt", bufs=2)
                nc.scalar.activation(lvt[:], lv[c][:], AF.Identity, accum_out=slv[:])
                lt = dec.tile([128, 1], f32, name=f"lt{c}", tag=f"lt{c}", bufs=1)
                nc.vector.tensor_tensor(lt[:], slv[:], smu2[:], OP.subtract)
                nc.vector.tensor_tensor(lt[:], lt[:], sstd2[:], OP.subtract)
                lsums.append(lt)
            stot = dec.tile([128, 1], f32, name="stot", tag="stot", bufs=1)
            nc.vector.tensor_tensor(stot[:], lsums[0][:], lsums[1][:], OP.add)
            nc.vector.tensor_tensor(stot[:], stot[:], lsums[2][:], OP.add)
            nc.vector.tensor_tensor(stot[:], stot[:], lsums[3][:], OP.add)
            psl = xpsum.tile([1, 1], f32, name="psl", tag="ps_x")
            nc.tensor.matmul(psl[:], stot[:], pcol[:, C_ONES:C_ONES + 1],
                             start=True, stop=True)
            lossv = dec.tile([1, 1], f32, name="lossv", tag="lossv", bufs=1)
            nc.scalar.activation(lossv[:], psl[:], AF.Identity,
                                 scale=-1.0 / (2.0 * BS),
                                 bias=pcol[0:1, C_N256:C_N256 + 1])
            nc.sync.dma_start(d_loss[:], lossv[:])
            zbm = dec.tile([BSH, H], f32, name="zbm", tag="zbm", bufs=1)
            for c in range(HC):
                pzt = xpsum.tile([BSH, 128], f32, name="pzt", tag="ps_x")
                nc.tensor.transpose(pzt[:], zT[c][:], ident[:])
                nc.vector.tensor_copy(zbm[:, 128 * c:128 * (c + 1)], pzt[:])
            nc.sync.dma_start(d_z[:], zbm[:])


    nc.finalize()
    return nc


def _host_prep(x, eps, enc_w1, enc_b1, bn1_g, bn1_b, enc_w2, enc_b2, bn2_g, bn2_b,
               enc_w3, enc_b3, gru_w_ih, gru_w_hh, gru_b_ih, gru_b_hh, fc_w, fc_b,
               k_binomial, k_activity):
    """Build per-core input maps (all numpy, f32)."""
    f = np.float32
    x = np.asarray(x, f)
    xs = x[:, 0, :T]                      # [512, 256] (also the encoder input)
    xt_base = np.ascontiguousarray(xs.T)  # [256, 512]
    whhT = np.ascontiguousarray(np.asarray(gru_w_hh, f).T)  # [512, 1536]
    w_ih = np.asarray(gru_w_ih, f)[:, 0]  # [1536]
    b_ih = np.asarray(gru_b_ih, f)
    b_hh = np.asarray(gru_b_hh, f)
    bsum = np.concatenate([(b_ih + b_hh)[:2 * H], b_hh[2 * H:]]).astype(f)

    # AddBeta weights + fc_b
    idx = np.arange(T)
    base = np.where(idx % 2 == 0, 0.5, 0.2).astype(f)
    l = T_CROSS / 5.0
    j = (idx // 4).astype(f)
    wb = np.exp(-((j - (T_CROSS - 1)) ** 2) / (2 * l * l))
    wa = np.exp(-(j ** 2) / (2 * l * l))
    off = base * np.where(idx % 4 < 2, wb, wa)
    eff = np.where(idx >= T_CROSS, -1.0, 1.0).astype(f)
    k = np.stack([np.asarray(k_binomial, f)[0], np.asarray(k_activity, f)[0]])
    wpb = (eff[None, :] * (np.logaddexp(0.0, k) + off[None, :])
           + np.asarray(fc_b, f)[:, None]).astype(f)        # [2, 256]

    pcol = np.zeros((128, NCOL), f)
    b3 = np.asarray(enc_b3, f)
    pcol[:100, C_B1] = np.asarray(enc_b1, f)
    pcol[:100, C_G1] = np.asarray(bn1_g, f)
    pcol[:100, C_BB1] = np.asarray(bn1_b, f)
    pcol[:100, C_B2] = np.asarray(enc_b2, f)
    pcol[:100, C_G2] = np.asarray(bn2_g, f)
    pcol[:100, C_BB2] = np.asarray(bn2_b, f)
    for c in range(HC):
        pcol[:, C_B3MU + c] = b3[128 * c:128 * (c + 1)]
        pcol[:, C_HB3LV + c] = 0.5 * b3[H + 128 * c:H + 128 * (c + 1)]
        pcol[:, C_B3LV + c] = b3[H + 128 * c:H + 128 * (c + 1)]
        pcol[:, C_FCW + 2 * c] = np.asarray(fc_w, f)[128 * c:128 * (c + 1), 0]
        pcol[:, C_FCW + 2 * c + 1] = np.asarray(fc_w, f)[128 * c:128 * (c + 1), 1]
    pcol[:, C_ONES] = 1.0
    pcol[:, C_EPS] = BN_EPS
    pcol[:, C_N256] = -float(H) / 2.0
    for c in range(HC):
        pcol[:, C_WR + c] = w_ih[128 * c:128 * (c + 1)]
        pcol[:, C_WZ + c] = w_ih[H + 128 * c:H + 128 * (c + 1)]
        pcol[:, C_WN + c] = w_ih[2 * H + 128 * c:2 * H + 128 * (c + 1)]
        pcol[:, C_BIHN + c] = b_ih[2 * H + 128 * c:2 * H + 128 * (c + 1)]
    pcol16 = np.zeros((128, 2 * HC), np.float16)
    for c in range(HC):
        pcol16[:, 2 * c] = np.asarray(fc_w, f)[128 * c:128 * (c + 1), 0]
        pcol16[:, 2 * c + 1] = np.asarray(fc_w, f)[128 * c:128 * (c + 1), 1]

    ident = np.eye(128, dtype=f)
    eps = np.asarray(eps, f)

    in_maps = []
    for ci in range(NCORES):
        b0 = ci * BSH
        prow = np.zeros((2, NROW), f)
        for rep in range(4):
            prow[0, WPB_OFF + T * rep:WPB_OFF + T * (rep + 1)] = wpb[0]
            prow[1, WPB_OFF + T * rep:WPB_OFF + T * (rep + 1)] = wpb[1]
        prow[0, BSUM_OFF:BSUM_OFF + 3 * H] = bsum
        prow[0, ONES_OFF:ONES_OFF + BSH] = 1.0
        iw16 = np.zeros((2, 2 + 4 * T), np.float16)
        iw16[0, 0] = 1.0
        iw16[1, 1] = 1.0
        for rep in range(4):
            iw16[0, 2 + T * rep:2 + T * (rep + 1)] = wpb[0]
            iw16[1, 2 + T * rep:2 + T * (rep + 1)] = wpb[1]
        in_maps.append({
            "iw16": iw16,
            "prow16": np.concatenate([xs[b0:b0 + BSH].reshape(-1), np.ones(128, f)]).reshape(1, -1).astype(np.float16),
            "pcol16": pcol16,
            "xt_full": np.ascontiguousarray(np.roll(xt_base, -b0, axis=1)),
            "epsT": np.ascontiguousarray(eps[b0:b0 + BSH].T),
            "whhT": whhT,
            "w1": np.asarray(enc_w1, f),
            "w2": np.asarray(enc_w2, f),
            "w3": np.asarray(enc_w3, f),
            "prow": prow,
            "pcol": pcol,
            "ident": ident,
        })
    return in_maps


_NC_CACHE = None
TRACE = False
LAST_RESULT = None


def kernel(**inputs):
    global _NC_CACHE, LAST_RESULT
    from concourse.bass_utils import run_bass_kernel_spmd

    in_maps = _host_prep(**inputs)
    if _NC_CACHE is None:
        _NC_CACHE = _build_program()
    nc = _NC_CACHE
    res = run_bass_kernel_spmd(nc, in_maps, list(range(NCORES)), trace=TRACE)
    LAST_RESULT = res
    results = res.results

    predict = np.zeros((BS, 1, 2, T), np.float32)
    z = np.zeros((BS, H), np.float32)
    for ci in range(NCORES):
        b0 = ci * BSH
        pred = np.asarray(results[ci]["out_pred"]).astype(np.float32)  # [32, 1024]
        # rows (2g, 2g+1) hold [c, (b0..b3 t)] for the quad
        pred = pred.reshape(BSH // 4, 2, 4, T).transpose(0, 2, 1, 3).reshape(BSH, 2, T)
        predict[b0:b0 + BSH, 0] = pred
        z[b0:b0 + BSH] = np.asarray(results[ci]["out_z"])
    loss = np.float32(np.asarray(results[0]["out_loss"]).reshape(-1)[0])
    return (predict, loss, z)
